# revision 6
# baseline (speedup 1.0000x reference)
"""Bahdanau attention kernel for Trainium2, data-parallel over batch on 8 NeuronCores.

Reference computation (B=32, T=2048, H=1024):
    q = query @ Wq.T + bq ; k = key @ Wk.T + bk
    score = tanh(q + k) @ Wz.T + bz            # [B,T,1]
    score = where(t >= lengths[b], -1e9, score)
    att_weights = softmax(score, axis=1)       # [B,T,1]
    att_value = att_weights^T @ value          # [B,1,H]
    returns (att_value, att_weights)

Device strategy (per core, 4 batch slots):
  - Compute dtype is bf16 for the TensorEngine (fp32 matmul runs at 1/4
    throughput on TRN2; measured fp32r is no faster). The host converts
    query/key (transposed to [b,H,T] so H lands on SBUF partitions),
    value, and the projection weights to bf16; accumulation stays fp32 in
    PSUM and the whole softmax pipeline is fp32.
  - Sequence-length sparsity: positions t >= lengths[b] carry weight
    exactly 0, so fully-masked 128-token tiles are skipped. The program
    is specialized on the per-slot tile counts (ceil(len/128)); batches
    are assigned to cores by sorted dealing so all 8 cores share one
    SPMD program (slot count = max over its rank group) and the load is
    balanced. Score columns of skipped tiles are pre-set to -1e9, so
    exp underflows to exactly 0 like the reference.
  - Per 128-token tile: accumulate q+k into PSUM (contract H in 8 chunks
    of 128; output o in 2 halves of 512), DVE adds (bq+bk), ACT tanh,
    DVE multiplies by Wz with a fused free-dim sum -> score column; the
    additive sequence mask is folded into the column combine.
  - Per batch: exp (no max subtraction: |score| <= 33 so exp is safe in
    fp32, and masked entries underflow to exactly 0), total via a
    ones-matmul, reciprocal, PE-transpose of the exp matrix for
    row-layout att_weights, and att_value = (w @ value) on PE
    contracting T, scaled by 1/sum.
  - bz is dropped: softmax is invariant to a constant shift of all
    unmasked scores, and bz is not otherwise observable in the outputs.
  - Software-pipelined one slot ahead so PE never waits on the DVE/ACT
    softmax tail.
"""

import os
import sys
import numpy as np
from contextlib import ExitStack

for _p in ("/opt/trn_rl_repo", "/root/.axon_site/_ro/trn_rl_repo"):
    if os.path.isdir(_p) and _p not in sys.path:
        sys.path.append(_p)

import ml_dtypes  # noqa: E402
import concourse.bass as bass  # noqa: E402
import concourse.tile as tile  # noqa: E402
from concourse import bacc, mybir  # noqa: E402
from concourse.bass_utils import run_bass_kernel_spmd  # noqa: E402

F32 = mybir.dt.float32
BF16 = mybir.dt.bfloat16
AF = mybir.ActivationFunctionType
ALU = mybir.AluOpType
BF16_NP = ml_dtypes.bfloat16

N_CORES = 8
B_FULL, T, H = 32, 2048, 1024
B_SH = B_FULL // N_CORES          # batch slots per core
P = 128                           # partitions
NT = T // P                       # 16 token tiles per batch
HC = H // P                       # 8 contraction chunks
NEG = -1e9

_nc_cache = {}
_last_results = None
_last_run_s = None


def _build(counts, repeat=1):
    """Emit the SPMD program for per-slot active tile counts (len B_SH)."""
    counts = tuple(int(c) for c in counts)
    assert len(counts) == B_SH and all(1 <= c <= NT for c in counts)

    nc = bacc.Bacc("TRN2", target_bir_lowering=False, debug=False)

    qT = nc.dram_tensor("qT", (B_SH, H, T), BF16, kind="ExternalInput").ap()
    kT = nc.dram_tensor("kT", (B_SH, H, T), BF16, kind="ExternalInput").ap()
    v = nc.dram_tensor("v", (B_SH, T, H), BF16, kind="ExternalInput").ap()
    wqT = nc.dram_tensor("wqT", (H, H), BF16, kind="ExternalInput").ap()
    wkT = nc.dram_tensor("wkT", (H, H), BF16, kind="ExternalInput").ap()
    bias2d = nc.dram_tensor("bias2d", (P, H), F32, kind="ExternalInput").ap()
    wz2d = nc.dram_tensor("wz2d", (P, H), F32, kind="ExternalInput").ap()
    maskcol = nc.dram_tensor("maskcol", (P, B_SH, NT), F32, kind="ExternalInput").ap()
    ident = nc.dram_tensor("ident", (P, P), F32, kind="ExternalInput").ap()

    attw = nc.dram_tensor("attw", (B_SH, NT, P), F32, kind="ExternalOutput").ap()
    av = nc.dram_tensor("av", (B_SH, 1, H), F32, kind="ExternalOutput").ap()

    with tile.TileContext(nc) as tc, ExitStack() as ctx:
        consts = ctx.enter_context(tc.tile_pool(name="consts", bufs=1))
        xq_pool = ctx.enter_context(tc.tile_pool(name="xq", bufs=3))
        xk_pool = ctx.enter_context(tc.tile_pool(name="xk", bufs=3))
        vt_pool = ctx.enter_context(tc.tile_pool(name="vt", bufs=3))
        scr_pool = ctx.enter_context(tc.tile_pool(name="scr", bufs=3))
        sm_pool = ctx.enter_context(tc.tile_pool(name="sm", bufs=3))
        qk_ps = ctx.enter_context(tc.tile_pool(name="qkps", bufs=2, space="PSUM"))
        av_ps = ctx.enter_context(tc.tile_pool(name="avps", bufs=1, space="PSUM"))
        sm_ps = ctx.enter_context(tc.tile_pool(name="smps", bufs=1, space="PSUM"))

        # ---- constants (weights chunk-split so the first matmuls start early) ----
        wq_sb = consts.tile([P, HC, H], BF16, tag="wq")
        wk_sb = consts.tile([P, HC, H], BF16, tag="wk")
        wq_r = wqT.rearrange("(c p) o -> p c o", p=P)
        wk_r = wkT.rearrange("(c p) o -> p c o", p=P)
        for c in range(HC):
            nc.sync.dma_start(wq_sb[:, c], wq_r[:, c])
            nc.sync.dma_start(wk_sb[:, c], wk_r[:, c])
        bias_sb = consts.tile([P, H], F32, tag="bias")
        nc.sync.dma_start(bias_sb[:], bias2d[:])
        wz_sb = consts.tile([P, H], F32, tag="wz")
        nc.sync.dma_start(wz_sb[:], wz2d[:])
        mask_sb = consts.tile([P, B_SH, NT], F32, tag="mask")
        nc.sync.dma_start(mask_sb[:], maskcol[:])
        ident_sb = consts.tile([P, P], F32, tag="ident")
        nc.sync.dma_start(ident_sb[:], ident[:])
        ones_col = consts.tile([P, 1], F32, tag="ones_col")
        nc.vector.memset(ones_col[:], 1.0)
        ones_row = consts.tile([1, P], F32, tag="ones_row")
        nc.vector.memset(ones_row[:], 1.0)

        st = [dict() for _ in range(B_SH)]  # per-slot live tiles

        def phase_A(b):
            """Projections + masked score columns for slot b (active tiles only)."""
            count = counts[b]
            sc = sm_pool.tile([P, NT], F32, tag="sc")
            st[b]["sc"] = sc
            if count < NT:
                nc.vector.memset(sc[:], NEG)
            for tg in range((count + 3) // 4):  # 512-token groups
                xq = xq_pool.tile([P, HC, 512], BF16, tag="xq")
                nc.sync.dma_start(
                    xq[:],
                    qT[b].rearrange("(c p) t -> p c t", p=P)[:, :, tg * 512:(tg + 1) * 512],
                )
                xk = xk_pool.tile([P, HC, 512], BF16, tag="xk")
                nc.sync.dma_start(
                    xk[:],
                    kT[b].rearrange("(c p) t -> p c t", p=P)[:, :, tg * 512:(tg + 1) * 512],
                )
                for jj in range(4):  # 128-token tiles within the group
                    j = tg * 4 + jj
                    if j >= count:
                        break
                    tsl = slice(jj * P, (jj + 1) * P)
                    acc0 = scr_pool.tile([P, 1], F32, tag="acc0")
                    acc1 = scr_pool.tile([P, 1], F32, tag="acc1")
                    for half, hacc in ((0, acc0), (1, acc1)):
                        osl = slice(half * 512, (half + 1) * 512)
                        pq = qk_ps.tile([P, 512], F32, tag="qk")
                        for c in range(HC):
                            nc.tensor.matmul(
                                pq[:], xq[:, c, tsl], wq_sb[:, c, osl],
                                start=(c == 0), stop=False,
                            )
                        for c in range(HC):
                            nc.tensor.matmul(
                                pq[:], xk[:, c, tsl], wk_sb[:, c, osl],
                                start=False, stop=(c == HC - 1),
                            )
                        qkb = scr_pool.tile([P, 512], F32, tag="qkb")
                        nc.vector.tensor_add(qkb[:], pq[:], bias_sb[:, osl])
                        tnh = scr_pool.tile([P, 512], F32, tag="tnh")
                        nc.scalar.activation(tnh[:], qkb[:], AF.Tanh)
                        prod = scr_pool.tile([P, 512], F32, tag="prod")
                        nc.vector.scalar_tensor_tensor(
                            out=prod[:], in0=tnh[:], scalar=1.0, in1=wz_sb[:, osl],
                            op0=ALU.mult, op1=ALU.mult, accum_out=hacc[:],
                        )
                    # sc[:, j] = (acc0 + mask) + acc1
                    nc.vector.scalar_tensor_tensor(
                        out=sc[:, j:j + 1], in0=acc0[:], scalar=mask_sb[:, b, j:j + 1],
                        in1=acc1[:], op0=ALU.add, op1=ALU.add,
                    )

        def phase_S_act(b):
            """exp of masked scores (+ per-partition row sums), bf16 copy."""
            e_f = sm_pool.tile([P, NT], F32, tag="e_f")
            acc = sm_pool.tile([P, 1], F32, tag="acc")
            nc.scalar.activation(e_f[:], st[b]["sc"][:], AF.Exp, accum_out=acc[:])
            e_h = sm_pool.tile([P, NT], BF16, tag="e_h")
            nc.vector.tensor_copy(e_h[:], e_f[:])
            st[b]["e_f"] = e_f
            st[b]["e_h"] = e_h
            st[b]["acc"] = acc

        def phase_S_rest(b):
            """Total sum -> 1/sum -> att_weights output."""
            s1 = sm_ps.tile([1, 1], F32, tag="s1")
            nc.tensor.matmul(s1[:], st[b]["acc"][:], ones_col[:], start=True, stop=True)
            r_sb = sm_pool.tile([1, 1], F32, tag="r_sb")
            nc.vector.reciprocal(r_sb[:], s1[:])
            st[b]["r_sb"] = r_sb
            rb = sm_ps.tile([P, 1], F32, tag="s1")  # reuses the s1 bank slot
            nc.tensor.matmul(rb[:], ones_row[:], r_sb[:], start=True, stop=True)
            rb_sb = sm_pool.tile([P, 1], F32, tag="rb_sb")
            nc.vector.tensor_copy(rb_sb[:], rb[:])
            et = sm_ps.tile([NT, P], F32, tag="et")
            nc.tensor.transpose(et[:], st[b]["e_f"][:], ident_sb[:])
            attw_sb = sm_pool.tile([NT, P], F32, tag="attw_sb")
            nc.vector.tensor_scalar_mul(attw_sb[:], et[:], rb_sb[0:NT])
            nc.sync.dma_start(attw[b], attw_sb[:])

        def phase_B(b):
            """att_value = (exp-weights @ value) / sum, active tiles only."""
            count = counts[b]
            e_h = st[b]["e_h"]
            av_lo = av_ps.tile([1, 512], F32, tag="av_lo")
            av_hi = av_ps.tile([1, 512], F32, tag="av_hi")
            for vg in range((count + 1) // 2):  # 2 token tiles per DMA
                vt = vt_pool.tile([P, 2, H], BF16, tag="vt")
                nc.sync.dma_start(
                    vt[:],
                    v[b].rearrange("(j p) h -> p j h", p=P)[:, vg * 2:(vg + 1) * 2, :],
                )
                for i in range(2):
                    j = vg * 2 + i
                    if j >= count:
                        break
                    for half, ps in ((0, av_lo), (1, av_hi)):
                        nc.tensor.matmul(
                            ps[:], e_h[:, j:j + 1], vt[:, i, half * 512:(half + 1) * 512],
                            start=(j == 0), stop=(j == count - 1),
                        )
            av_sb = sm_pool.tile([1, H], F32, tag="av_sb")
            r_sb = st[b]["r_sb"]
            nc.vector.tensor_scalar_mul(av_sb[:, 0:512], av_lo[:], r_sb[0:1])
            nc.vector.tensor_scalar_mul(av_sb[:, 512:H], av_hi[:], r_sb[0:1])
            nc.sync.dma_start(av[b], av_sb[:])

        # Software-pipelined schedule: PE work of slot b+1 is enqueued before
        # the softmax/value phases of slot b, so PE never drains.
        # KERNEL_PHASES trims the emitted program for debugging; repeat>1
        # re-emits the whole pipeline for per-iteration timing.
        phases = os.environ.get("KERNEL_PHASES", "full")
        for _ in range(repeat):
            if phases == "A":
                for b in range(B_SH):
                    phase_A(b)
            elif phases == "AS":
                for b in range(B_SH):
                    phase_A(b)
                    phase_S_act(b)
            else:
                phase_A(0)
                phase_S_act(0)
                phase_A(1)
                phase_S_act(1)
                phase_S_rest(0)
                phase_B(0)
                phase_A(2)
                phase_S_act(2)
                phase_S_rest(1)
                phase_B(1)
                phase_A(3)
                phase_S_act(3)
                phase_S_rest(2)
                phase_B(2)
                phase_S_rest(3)
                phase_B(3)

    nc.compile()
    return nc


def _get_nc(counts):
    key = (tuple(counts), int(os.environ.get("KERNEL_REPEAT", "1")))
    if key not in _nc_cache:
        _nc_cache[key] = _build(key[0], repeat=key[1])
    return _nc_cache[key]


def _plan(lengths):
    """Batch->core/slot assignment and per-slot tile counts.

    Sorted dealing: rank batches by tile count (desc); rank group s
    (ranks 8s..8s+7) becomes slot s, one batch per core. The slot's
    compiled tile count is the group max; the mask zeroes the excess.
    Returns (perm[B_SH][N_CORES] batch indices, counts[B_SH]).
    """
    tiles = np.minimum((np.asarray(lengths) + P - 1) // P, NT).astype(np.int64)
    tiles = np.maximum(tiles, 1)
    order = np.argsort(-tiles, kind="stable")
    perm = order.reshape(B_SH, N_CORES)
    counts = [int(tiles[perm[s]].max()) for s in range(B_SH)]
    return perm, counts


def _prep_host(inputs):
    query = np.asarray(inputs["query"], dtype=np.float32)
    key = inputs.get("key_", inputs.get("key"))
    key = np.asarray(key, dtype=np.float32)
    value = np.asarray(inputs["value"], dtype=np.float32)
    lengths = np.asarray(inputs["lengths"]).astype(np.int64)
    Wq = np.asarray(inputs["Wq"], dtype=np.float32)
    bq = np.asarray(inputs["bq"], dtype=np.float32)
    Wk = np.asarray(inputs["Wk"], dtype=np.float32)
    bk = np.asarray(inputs["bk"], dtype=np.float32)
    Wz = np.asarray(inputs["Wz"], dtype=np.float32)

    assert query.shape == (B_FULL, T, H), query.shape

    qT = np.ascontiguousarray(query.transpose(0, 2, 1)).astype(BF16_NP)
    kT = np.ascontiguousarray(key.transpose(0, 2, 1)).astype(BF16_NP)
    vh = value.astype(BF16_NP)
    wqT = np.ascontiguousarray(Wq.T).astype(BF16_NP)
    wkT = np.ascontiguousarray(Wk.T).astype(BF16_NP)
    bias2d = np.ascontiguousarray(
        np.broadcast_to((bq + bk)[None, :], (P, H)), dtype=np.float32
    )
    wz2d = np.ascontiguousarray(np.broadcast_to(Wz[0][None, :], (P, H)),
                                dtype=np.float32)
    ident = np.eye(P, dtype=np.float32)

    t_idx = np.arange(T)
    maskbias = np.where(t_idx[None, :] >= lengths[:, None], np.float32(NEG),
                        np.float32(0.0)).astype(np.float32)  # [B, T]
    maskcol_all = maskbias.reshape(B_FULL, NT, P).transpose(0, 2, 1)  # [B, P, NT]

    perm, counts = _plan(lengths)

    in_maps = []
    for c in range(N_CORES):
        bidx = perm[:, c]  # batch index per slot for this core
        in_maps.append({
            "qT": qT[bidx],
            "kT": kT[bidx],
            "v": vh[bidx],
            "wqT": wqT,
            "wkT": wkT,
            "bias2d": bias2d,
            "wz2d": wz2d,
            "maskcol": np.ascontiguousarray(
                maskcol_all[bidx].transpose(1, 0, 2)),
            "ident": ident,
        })
    return in_maps, perm, counts


def kernel(**inputs):
    global _last_results, _last_run_s
    import time as _time

    in_maps, perm, counts = _prep_host(inputs)
    nc = _get_nc(counts)

    t0 = _time.time()
    res = run_bass_kernel_spmd(nc, in_maps, core_ids=list(range(N_CORES)))
    _last_run_s = _time.time() - t0
    _last_results = res

    att_value = np.empty((B_FULL, 1, H), dtype=np.float32)
    att_weights = np.empty((B_FULL, T, 1), dtype=np.float32)
    for c in range(N_CORES):
        r = res.results[c]
        for s in range(B_SH):
            b = perm[s, c]
            att_value[b, 0, :] = r["av"][s, 0, :]
            att_weights[b, :, 0] = r["attw"][s].reshape(T)
    return att_value, att_weights


# revision 12
# speedup vs baseline: 1.1876x; 1.1876x over previous
"""Bahdanau attention kernel for Trainium2, data-parallel over batch on 8 NeuronCores.

Reference computation (B=32, T=2048, H=1024):
    q = query @ Wq.T + bq ; k = key @ Wk.T + bk
    score = tanh(q + k) @ Wz.T + bz            # [B,T,1]
    score = where(t >= lengths[b], -1e9, score)
    att_weights = softmax(score, axis=1)       # [B,T,1]
    att_value = att_weights^T @ value          # [B,1,H]
    returns (att_value, att_weights)

Device strategy (per core, 4 batch slots):
  - Compute dtype is bf16 for the TensorEngine (fp32 matmul runs at 1/4
    throughput on TRN2; measured fp32r is no faster). The host converts
    query/key (transposed to [b,H,T] so H lands on SBUF partitions),
    value, and the projection weights to bf16; accumulation stays fp32 in
    PSUM and the whole softmax pipeline is fp32.
  - Sequence-length sparsity: positions t >= lengths[b] carry weight
    exactly 0, so fully-masked 128-token tiles are skipped. The program
    is specialized on the per-slot tile counts (ceil(len/128)); batches
    are assigned to cores by sorted dealing so all 8 cores share one
    SPMD program (slot count = max over its rank group) and the load is
    balanced. Score columns of skipped tiles are pre-set to -1e9, so
    exp underflows to exactly 0 like the reference.
  - Per 128-token tile: accumulate q+k into PSUM (contract H in 8 chunks
    of 128; output o in 2 halves of 512), DVE adds (bq+bk), ACT tanh,
    DVE multiplies by Wz with a fused free-dim sum -> score column; the
    additive sequence mask is folded into the column combine.
  - Per batch: exp (no max subtraction: |score| <= 33 so exp is safe in
    fp32, and masked entries underflow to exactly 0), total via a
    ones-matmul, reciprocal, PE-transpose of the exp matrix for
    row-layout att_weights, and att_value = (w @ value) on PE
    contracting T, scaled by 1/sum.
  - bz is dropped: softmax is invariant to a constant shift of all
    unmasked scores, and bz is not otherwise observable in the outputs.
  - Software-pipelined one slot ahead so PE never waits on the DVE/ACT
    softmax tail.
"""

import os
import sys
import numpy as np
from contextlib import ExitStack

for _p in ("/opt/trn_rl_repo", "/root/.axon_site/_ro/trn_rl_repo"):
    if os.path.isdir(_p) and _p not in sys.path:
        sys.path.append(_p)

import ml_dtypes  # noqa: E402
import concourse.bass as bass  # noqa: E402
import concourse.tile as tile  # noqa: E402
from concourse import bacc, mybir  # noqa: E402
from concourse.bass_utils import run_bass_kernel_spmd  # noqa: E402

F32 = mybir.dt.float32
BF16 = mybir.dt.bfloat16
AF = mybir.ActivationFunctionType
ALU = mybir.AluOpType
BF16_NP = ml_dtypes.bfloat16

N_CORES = 8
B_FULL, T, H = 32, 2048, 1024
B_SH = B_FULL // N_CORES          # batch slots per core
P = 128                           # partitions
NT = T // P                       # 16 token tiles per batch
HC = H // P                       # 8 contraction chunks
NEG = -1e9

_nc_cache = {}
_last_results = None
_last_run_s = None


def _build(counts, repeat=1):
    """Emit the SPMD program for per-slot active tile counts (len B_SH)."""
    counts = tuple(int(c) for c in counts)
    assert len(counts) == B_SH and all(1 <= c <= NT for c in counts)

    nc = bacc.Bacc("TRN2", target_bir_lowering=False, debug=False)

    # Per-slot tensors sized to the active length prefix (counts[s]*128
    # tokens) — masked tails are never uploaded or read.
    qT = [nc.dram_tensor(f"qT{s}", (H, counts[s] * P), BF16,
                         kind="ExternalInput").ap() for s in range(B_SH)]
    kT = [nc.dram_tensor(f"kT{s}", (H, counts[s] * P), BF16,
                         kind="ExternalInput").ap() for s in range(B_SH)]
    v = [nc.dram_tensor(f"v{s}", (counts[s] * P, H), BF16,
                        kind="ExternalInput").ap() for s in range(B_SH)]
    wqT = nc.dram_tensor("wqT", (H, H), BF16, kind="ExternalInput").ap()
    wkT = nc.dram_tensor("wkT", (H, H), BF16, kind="ExternalInput").ap()
    bias2d = nc.dram_tensor("bias2d", (P, H), F32, kind="ExternalInput").ap()
    wz2d = nc.dram_tensor("wz2d", (P, H), F32, kind="ExternalInput").ap()
    maskcol = nc.dram_tensor("maskcol", (P, B_SH, NT), F32, kind="ExternalInput").ap()
    ident = nc.dram_tensor("ident", (P, P), F32, kind="ExternalInput").ap()

    attw = nc.dram_tensor("attw", (B_SH, NT, P), F32, kind="ExternalOutput").ap()
    av = nc.dram_tensor("av", (B_SH, 1, H), F32, kind="ExternalOutput").ap()

    with tile.TileContext(nc) as tc, ExitStack() as ctx:
        consts = ctx.enter_context(tc.tile_pool(name="consts", bufs=1))
        xq_pool = ctx.enter_context(tc.tile_pool(name="xq", bufs=3))
        xk_pool = ctx.enter_context(tc.tile_pool(name="xk", bufs=3))
        vt_pool = ctx.enter_context(tc.tile_pool(name="vt", bufs=3))
        scr_pool = ctx.enter_context(tc.tile_pool(name="scr", bufs=3))
        sm_pool = ctx.enter_context(tc.tile_pool(name="sm", bufs=3))
        qk_ps = ctx.enter_context(tc.tile_pool(name="qkps", bufs=2, space="PSUM"))
        av_ps = ctx.enter_context(tc.tile_pool(name="avps", bufs=1, space="PSUM"))
        sm_ps = ctx.enter_context(tc.tile_pool(name="smps", bufs=1, space="PSUM"))

        # ---- constants (weights chunk-split so the first matmuls start early) ----
        wq_sb = consts.tile([P, HC, H], BF16, tag="wq")
        wk_sb = consts.tile([P, HC, H], BF16, tag="wk")
        wq_r = wqT.rearrange("(c p) o -> p c o", p=P)
        wk_r = wkT.rearrange("(c p) o -> p c o", p=P)
        for c in range(HC):
            nc.sync.dma_start(wq_sb[:, c], wq_r[:, c])
            nc.sync.dma_start(wk_sb[:, c], wk_r[:, c])
        bias_sb = consts.tile([P, H], F32, tag="bias")
        nc.sync.dma_start(bias_sb[:], bias2d[:])
        wz_sb = consts.tile([P, H], F32, tag="wz")
        nc.sync.dma_start(wz_sb[:], wz2d[:])
        mask_sb = consts.tile([P, B_SH, NT], F32, tag="mask")
        nc.sync.dma_start(mask_sb[:], maskcol[:])
        ident_sb = consts.tile([P, P], F32, tag="ident")
        nc.sync.dma_start(ident_sb[:], ident[:])
        ones_col = consts.tile([P, 1], F32, tag="ones_col")
        nc.vector.memset(ones_col[:], 1.0)
        ones_row = consts.tile([1, P], F32, tag="ones_row")
        nc.vector.memset(ones_row[:], 1.0)

        st = [dict() for _ in range(B_SH)]  # per-slot live tiles

        def phase_A(b):
            """Projections + masked score columns for slot b (active tiles only)."""
            count = counts[b]
            sc = sm_pool.tile([P, NT], F32, tag="sc")
            st[b]["sc"] = sc
            if count < NT:
                nc.vector.memset(sc[:], NEG)
            for tg in range((count + 3) // 4):  # 512-token groups
                w = min(512, count * P - tg * 512)  # last group may be partial
                xq = xq_pool.tile([P, HC, 512], BF16, tag="xq")
                nc.sync.dma_start(
                    xq[:, :, 0:w],
                    qT[b].rearrange("(c p) t -> p c t", p=P)[:, :, tg * 512:tg * 512 + w],
                )
                xk = xk_pool.tile([P, HC, 512], BF16, tag="xk")
                nc.sync.dma_start(
                    xk[:, :, 0:w],
                    kT[b].rearrange("(c p) t -> p c t", p=P)[:, :, tg * 512:tg * 512 + w],
                )
                for jj in range(4):  # 128-token tiles within the group
                    j = tg * 4 + jj
                    if j >= count:
                        break
                    tsl = slice(jj * P, (jj + 1) * P)
                    acc0 = scr_pool.tile([P, 1], F32, tag="acc0")
                    acc1 = scr_pool.tile([P, 1], F32, tag="acc1")
                    for half, hacc in ((0, acc0), (1, acc1)):
                        osl = slice(half * 512, (half + 1) * 512)
                        pq = qk_ps.tile([P, 512], F32, tag="qk")
                        for c in range(HC):
                            nc.tensor.matmul(
                                pq[:], xq[:, c, tsl], wq_sb[:, c, osl],
                                start=(c == 0), stop=False,
                            )
                        for c in range(HC):
                            nc.tensor.matmul(
                                pq[:], xk[:, c, tsl], wk_sb[:, c, osl],
                                start=False, stop=(c == HC - 1),
                            )
                        qkb = scr_pool.tile([P, 512], F32, tag="qkb")
                        nc.vector.tensor_add(qkb[:], pq[:], bias_sb[:, osl])
                        tnh = scr_pool.tile([P, 512], F32, tag="tnh")
                        nc.scalar.activation(tnh[:], qkb[:], AF.Tanh)
                        prod = scr_pool.tile([P, 512], F32, tag="prod")
                        nc.vector.scalar_tensor_tensor(
                            out=prod[:], in0=tnh[:], scalar=1.0, in1=wz_sb[:, osl],
                            op0=ALU.mult, op1=ALU.mult, accum_out=hacc[:],
                        )
                    # sc[:, j] = (acc0 + mask) + acc1
                    nc.vector.scalar_tensor_tensor(
                        out=sc[:, j:j + 1], in0=acc0[:], scalar=mask_sb[:, b, j:j + 1],
                        in1=acc1[:], op0=ALU.add, op1=ALU.add,
                    )

        def phase_S_act(b):
            """exp of masked scores (+ per-partition row sums), bf16 copy."""
            e_f = sm_pool.tile([P, NT], F32, tag="e_f")
            acc = sm_pool.tile([P, 1], F32, tag="acc")
            nc.scalar.activation(e_f[:], st[b]["sc"][:], AF.Exp, accum_out=acc[:])
            e_h = sm_pool.tile([P, NT], BF16, tag="e_h")
            nc.vector.tensor_copy(e_h[:], e_f[:])
            st[b]["e_f"] = e_f
            st[b]["e_h"] = e_h
            st[b]["acc"] = acc

        def phase_S_rest(b):
            """Total sum -> 1/sum -> att_weights output."""
            s1 = sm_ps.tile([1, 1], F32, tag="s1")
            nc.tensor.matmul(s1[:], st[b]["acc"][:], ones_col[:], start=True, stop=True)
            r_sb = sm_pool.tile([1, 1], F32, tag="r_sb")
            nc.vector.reciprocal(r_sb[:], s1[:])
            st[b]["r_sb"] = r_sb
            rb = sm_ps.tile([P, 1], F32, tag="s1")  # reuses the s1 bank slot
            nc.tensor.matmul(rb[:], ones_row[:], r_sb[:], start=True, stop=True)
            rb_sb = sm_pool.tile([P, 1], F32, tag="rb_sb")
            nc.vector.tensor_copy(rb_sb[:], rb[:])
            et = sm_ps.tile([NT, P], F32, tag="et")
            nc.tensor.transpose(et[:], st[b]["e_f"][:], ident_sb[:])
            attw_sb = sm_pool.tile([NT, P], F32, tag="attw_sb")
            nc.vector.tensor_scalar_mul(attw_sb[:], et[:], rb_sb[0:NT])
            nc.sync.dma_start(attw[b], attw_sb[:])

        def phase_B(b):
            """att_value = (exp-weights @ value) / sum, active tiles only."""
            count = counts[b]
            e_h = st[b]["e_h"]
            av_lo = av_ps.tile([1, 512], F32, tag="av_lo")
            av_hi = av_ps.tile([1, 512], F32, tag="av_hi")
            for vg in range((count + 1) // 2):  # 2 token tiles per DMA
                w = min(2, count - vg * 2)  # last group may be a single tile
                vt = vt_pool.tile([P, 2, H], BF16, tag="vt")
                nc.sync.dma_start(
                    vt[:, 0:w],
                    v[b].rearrange("(j p) h -> p j h", p=P)[:, vg * 2:vg * 2 + w, :],
                )
                for i in range(2):
                    j = vg * 2 + i
                    if j >= count:
                        break
                    for half, ps in ((0, av_lo), (1, av_hi)):
                        nc.tensor.matmul(
                            ps[:], e_h[:, j:j + 1], vt[:, i, half * 512:(half + 1) * 512],
                            start=(j == 0), stop=(j == count - 1),
                        )
            av_sb = sm_pool.tile([1, H], F32, tag="av_sb")
            r_sb = st[b]["r_sb"]
            nc.vector.tensor_scalar_mul(av_sb[:, 0:512], av_lo[:], r_sb[0:1])
            nc.vector.tensor_scalar_mul(av_sb[:, 512:H], av_hi[:], r_sb[0:1])
            nc.sync.dma_start(av[b], av_sb[:])

        # Software-pipelined schedule: PE work of slot b+1 is enqueued before
        # the softmax/value phases of slot b, so PE never drains.
        # KERNEL_PHASES trims the emitted program for debugging; repeat>1
        # re-emits the whole pipeline for per-iteration timing.
        phases = os.environ.get("BK_PHASES", "full")
        for _ in range(repeat):
            if phases == "A":
                for b in range(B_SH):
                    phase_A(b)
            elif phases == "AS":
                for b in range(B_SH):
                    phase_A(b)
                    phase_S_act(b)
            else:
                phase_A(0)
                phase_S_act(0)
                phase_A(1)
                phase_S_act(1)
                phase_S_rest(0)
                phase_B(0)
                phase_A(2)
                phase_S_act(2)
                phase_S_rest(1)
                phase_B(1)
                phase_A(3)
                phase_S_act(3)
                phase_S_rest(2)
                phase_B(2)
                phase_S_rest(3)
                phase_B(3)

    nc.compile()
    return nc


def _get_nc(counts):
    key = (tuple(counts), int(os.environ.get("BK_REPEAT", "1")))
    if key not in _nc_cache:
        _nc_cache[key] = _build(key[0], repeat=key[1])
    return _nc_cache[key]


def _plan(lengths):
    """Batch->core/slot assignment and per-slot tile counts.

    Sorted dealing: rank batches by tile count (desc); rank group s
    (ranks 8s..8s+7) becomes slot s, one batch per core. The slot's
    compiled tile count is the group max; the mask zeroes the excess.
    Returns (perm[B_SH][N_CORES] batch indices, counts[B_SH]).
    """
    tiles = np.minimum((np.asarray(lengths) + P - 1) // P, NT).astype(np.int64)
    tiles = np.maximum(tiles, 1)
    order = np.argsort(-tiles, kind="stable")
    perm = order.reshape(B_SH, N_CORES)
    counts = [int(tiles[perm[s]].max()) for s in range(B_SH)]
    return perm, counts


def _prep_host(inputs):
    query = np.asarray(inputs["query"], dtype=np.float32)
    key = inputs.get("key_", inputs.get("key"))
    key = np.asarray(key, dtype=np.float32)
    value = np.asarray(inputs["value"], dtype=np.float32)
    lengths = np.asarray(inputs["lengths"]).astype(np.int64)
    Wq = np.asarray(inputs["Wq"], dtype=np.float32)
    bq = np.asarray(inputs["bq"], dtype=np.float32)
    Wk = np.asarray(inputs["Wk"], dtype=np.float32)
    bk = np.asarray(inputs["bk"], dtype=np.float32)
    Wz = np.asarray(inputs["Wz"], dtype=np.float32)

    assert query.shape == (B_FULL, T, H), query.shape

    wqT = np.ascontiguousarray(Wq.T).astype(BF16_NP)
    wkT = np.ascontiguousarray(Wk.T).astype(BF16_NP)
    bias2d = np.ascontiguousarray(
        np.broadcast_to((bq + bk)[None, :], (P, H)), dtype=np.float32
    )
    wz2d = np.ascontiguousarray(np.broadcast_to(Wz[0][None, :], (P, H)),
                                dtype=np.float32)
    ident = np.eye(P, dtype=np.float32)

    t_idx = np.arange(T)
    maskbias = np.where(t_idx[None, :] >= lengths[:, None], np.float32(NEG),
                        np.float32(0.0)).astype(np.float32)  # [B, T]
    maskcol_all = maskbias.reshape(B_FULL, NT, P).transpose(0, 2, 1)  # [B, P, NT]

    perm, counts = _plan(lengths)

    in_maps = []
    for c in range(N_CORES):
        bidx = perm[:, c]  # batch index per slot for this core
        m = {
            "wqT": wqT,
            "wkT": wkT,
            "bias2d": bias2d,
            "wz2d": wz2d,
            "maskcol": np.ascontiguousarray(
                maskcol_all[bidx].transpose(1, 0, 2)),
            "ident": ident,
        }
        for s in range(B_SH):
            b = int(bidx[s])
            tp = counts[s] * P  # active token prefix
            m[f"qT{s}"] = np.ascontiguousarray(
                query[b, :tp, :].T).astype(BF16_NP)
            m[f"kT{s}"] = np.ascontiguousarray(
                key[b, :tp, :].T).astype(BF16_NP)
            m[f"v{s}"] = value[b, :tp, :].astype(BF16_NP)
        in_maps.append(m)
    return in_maps, perm, counts


def kernel(**inputs):
    global _last_results, _last_run_s
    import time as _time

    in_maps, perm, counts = _prep_host(inputs)
    nc = _get_nc(counts)

    t0 = _time.time()
    res = run_bass_kernel_spmd(nc, in_maps, core_ids=list(range(N_CORES)))
    _last_run_s = _time.time() - t0
    _last_results = res

    att_value = np.empty((B_FULL, 1, H), dtype=np.float32)
    att_weights = np.empty((B_FULL, T, 1), dtype=np.float32)
    for c in range(N_CORES):
        r = res.results[c]
        for s in range(B_SH):
            b = perm[s, c]
            att_value[b, 0, :] = r["av"][s, 0, :]
            att_weights[b, :, 0] = r["attw"][s].reshape(T)
    return att_value, att_weights
